# revision 12
# baseline (speedup 1.0000x reference)
# Trainium2 Bass kernel for nn_CrossAttentionSpatialWeight.
#
# Row-sharded across 8 NeuronCores: core c owns query rows [c*512, (c+1)*512).
# Each core computes (redundantly) the full K^T / V / spatial-MLP features from
# the full inputs, then its 512x4096 block of logits
#   logits = Q K^T/16 + exp(-d2/2) + cos_sim + tanh(f_i - f_j)
# accumulated in PSUM (es + qk via bf16 matmuls; rank-1/2 terms via identity
# matmuls of ACT-produced tiles), then softmax (no max-subtract needed: |logits|
# <~ 6) and attn @ V via PE-transposed numerator blocks.
import os
import numpy as np
import ml_dtypes

N, G, H, O = 4096, 512, 256, 256
NCORES = 8
R = N // NCORES          # 512 own rows per core
OT = R // 128            # 4 own-row tiles
NT = N // 128            # 32 j-tiles
GC = G // 128            # 4 gene-dim chunks
HC = H // 128            # 2 hidden chunks
JC = 8                   # j-chunks per row-tile
JCW = N // JC            # 512


def _build_nc():
    import concourse.bacc as bacc
    import concourse.mybir as mybir
    import concourse.tile as tile

    dt = mybir.dt
    f32, bf, f16 = dt.float32, dt.bfloat16, dt.float16
    AF = mybir.ActivationFunctionType
    ALU = mybir.AluOpType

    nc = bacc.Bacc("TRN2", target_bir_lowering=False, debug=False,
                   num_devices=NCORES)

    def din(name, shape):
        return nc.dram_tensor(name, list(shape), f32, kind="ExternalInput").ap()

    gene = din("gene_expr", (N, G))
    pos = din("spatial_pos", (N, 2))
    gown = din("gene_own", (R, G))
    posown = din("pos_own", (R, 2))
    Wq, bq = din("Wq", (G, H)), din("bq", (H,))
    Wk, bk = din("Wk", (G, H)), din("bk", (H,))
    Wv, bv = din("Wv", (G, O)), din("bv", (O,))
    Ws, bs = din("Ws", (2, H)), din("bs", (H,))
    W1, b1 = din("W1", (H, H)), din("b1", (H,))
    W2, b2 = din("W2", (H, H)), din("b2", (H,))
    Wf, bf_b = din("Wf", (H, 1)), din("bf", (1,))
    ident = nc.dram_tensor("ident", [128, 128], bf, kind="ExternalInput").ap()

    attn_out = nc.dram_tensor("attn_own", [R, N], f32, kind="ExternalOutput").ap()
    upd_out = nc.dram_tensor("upd_own", [R, O], f32, kind="ExternalOutput").ap()

    xn_dram = nc.dram_tensor("xn_scratch", [N, G], bf, kind="Internal").ap()
    xno_dram = nc.dram_tensor("xno_scratch", [R, G], bf, kind="Internal").ap()
    rowscr_f = nc.dram_tensor("rowscr_f", [4, N], f32, kind="Internal").ap()
    rowscr_b = nc.dram_tensor("rowscr_b", [16, N], bf, kind="Internal").ap()

    with tile.TileContext(nc) as tc:
        act, vec, pe, sync, gps = nc.scalar, nc.vector, nc.tensor, nc.sync, nc.gpsimd

        _keep = []  # keep free-handles alive so GC can't close single pools

        def T(shape, dtype, name):
            t, free = tc.tile(shape, dtype, name=name)
            _keep.append(free)
            return t

        # ---------- persistent SBUF (~110 KB/partition) ----------
        idb = T([128, 128], bf, name="idb")
        sync.dma_start(idb[:], ident)

        xnT = T([128, GC * N], bf, name="xnT")        # 32K/p  xn^T chunks
        KT = T([128, HC * N], bf, name="KT")          # 16K/p  K^T chunks
        Vb = T([128, NT * O], bf, name="Vb")          # 16K/p  V (normal)
        xnoT = T([128, GC * R], bf, name="xnoT")      # own xn^T (lhsT)
        QT = T([128, HC * R], bf, name="QT")          # own Q^T/16
        spown = T([8, OT * 128], bf, name="spown")    # sp lhsT rows
        sprhs = T([8, N], bf, name="sprhs")           # sp rhs rows
        posr7 = T([7, N], bf, name="posr7")           # MLP si rhs rows
        posro7 = T([7, R], bf, name="posro7")
        WsT7 = T([7, HC * 128], bf, name="WsT7")      # MLP si lhsT rows
        fown = T([128, OT], f32, name="fown")
        nsqo = T([128, OT], f32, name="nsqo")         # -|pos_i|^2/2
        norm32 = T([128, NT], f32, name="norm32")
        rnorm32 = T([128, NT], f32, name="rnorm32")
        rnb32 = T([128, NT], bf, name="rnb32")
        normo = T([128, OT], f32, name="normo")
        rnormo = T([128, OT], f32, name="rnormo")
        rn_row = T([1, N], bf, name="rn_row")         # 1/norm_j row
        nbo = T([128, R], f32, name="nbo")            # norm_i bcast (own)
        bfbc = T([128, 1], f32, name="bfbc")
        ssq = T([128, NT], f32, name="ssq")
        ssqo = T([128, OT], f32, name="ssqo")
        possq = T([128, NT], f32, name="possq")
        posHL = T([128, NT * 4], bf, name="posHL")    # [xh yh xl yl]/jt
        posHLo = T([128, OT * 4], bf, name="posHLo")
        sh32 = T([128, NT], bf, name="sh32")          # hi(-|p_j|^2/2)
        sl32 = T([128, NT], bf, name="sl32")          # lo residual

        # weights (bf16/f16 casts)
        Wqb = T([128, GC * H], bf, name="Wqb")
        Wkb = T([128, GC * H], bf, name="Wkb")
        Wvb = T([128, GC * O], bf, name="Wvb")
        W1f = T([128, HC * H], f16, name="W1f")
        W2f = T([128, HC * H], f16, name="W2f")
        Wff = T([128, HC], f16, name="Wff")
        bq16 = T([128, HC], f32, name="bq16")
        bkb = T([1, H], bf, name="bkb")
        b1c = T([128, HC], f32, name="b1c")
        b2c = T([128, HC], f32, name="b2c")
        bvb = T([128, O], f32, name="bvb")

        with tc.tile_pool(name="wload", bufs=2) as wl:
            for W, Wc, n_out, scale in ((Wq, Wqb, H, 1.0 / 16.0),
                                        (Wk, Wkb, H, 1.0), (Wv, Wvb, O, 1.0)):
                t = wl.tile([128, GC * n_out], f32, tag="w")
                for gc in range(GC):
                    sync.dma_start(t[:, gc * n_out:(gc + 1) * n_out],
                                   W[gc * 128:(gc + 1) * 128, :])
                act.mul(Wc[:], t[:], scale)
            for W, Wc in ((W1, W1f), (W2, W2f)):
                t = wl.tile([128, HC * H], f32, tag="w")
                for kc in range(HC):
                    sync.dma_start(t[:, kc * H:(kc + 1) * H],
                                   W[kc * 128:(kc + 1) * 128, :])
                act.mul(Wc[:], t[:], 1.0)
            t = wl.tile([128, HC], f32, tag="small")
            sync.dma_start(t[:], Wf.rearrange("(c p) o -> p (c o)", p=128))
            act.mul(Wff[:], t[:], 1.0)
            # biases
            t = wl.tile([128, HC], f32, tag="small")
            sync.dma_start(t[:], bq.rearrange("(c p) -> p c", p=128))
            vec.tensor_scalar_mul(bq16[:], t[:], 1.0 / 16.0)
            sync.dma_start(b1c[:], b1.rearrange("(c p) -> p c", p=128))
            sync.dma_start(b2c[:], b2.rearrange("(c p) -> p c", p=128))
            t2 = wl.tile([1, H], f32, tag="row")
            sync.dma_start(t2[:], bk[None, :])
            vec.tensor_copy(bkb[:], t2[:])
            t3 = wl.tile([1, O], f32, tag="row")
            sync.dma_start(t3[:], bv[None, :])
            gps.partition_broadcast(bvb[:], t3[:])
            t4 = wl.tile([1, 1], f32, tag="tiny")
            sync.dma_start(t4[:], bf_b[None, :])
            gps.partition_broadcast(bfbc[:], t4[:])
            # Ws rows -> WsT7 (hi/lo split), pairing rhs=[xh,xh,xl,yh,yh,yl,1]
            # lhsT rows = [Wxh, Wxl, Wxh, Wyh, Wyl, Wyh, bs]
            tws = wl.tile([2, H], f32, tag="row2")
            sync.dma_start(tws[:], Ws[:, :])
            tbs = wl.tile([1, H], f32, tag="row")
            sync.dma_start(tbs[:], bs[None, :])
            whl = wl.tile([2, H], bf, tag="row2b")   # hi
            wlo = wl.tile([2, H], bf, tag="row2c")   # lo
            vec.tensor_copy(whl[:], tws[:])
            vec.tensor_sub(wlo[:], tws[:], whl[:])
            tbsb = wl.tile([1, H], bf, tag="rowb")
            vec.tensor_copy(tbsb[:], tbs[:])
            for r, srcrow in ((0, tbsb[0:1, :]), (1, whl[0:1, :]),
                              (2, wlo[0:1, :]), (3, whl[0:1, :]),
                              (4, whl[1:2, :]), (5, wlo[1:2, :]),
                              (6, whl[1:2, :])):
                gps.dma_start(WsT7[r:r + 1, :], srcrow)

        # ---------- P1: stream gene, norms, xn -> DRAM -> xn^T ----------
        with tc.tile_pool(name="p1g", bufs=3) as p1g, \
             tc.tile_pool(name="p1x", bufs=3) as p1x, \
             tc.tile_pool(name="p1s", bufs=3) as p1s:
            def handle_tile(idx, src_g, src_p, ssq_t, sq_t, hi_t, nrm_t, rnrm_t,
                            xdram):
                gt = p1g.tile([128, G], f32, tag="g")
                sync.dma_start(gt[:], src_g)
                scr = p1s.tile([128, G], f32, tag="scr")
                act.activation(scr[:], gt[:], AF.Square,
                               accum_out=ssq_t[:, idx:idx + 1])
                pt = p1s.tile([128, 2], f32, tag="pos")
                sync.dma_start(pt[:], src_p)
                pscr = p1s.tile([128, 2], f32, tag="pscr")
                act.activation(pscr[:], pt[:], AF.Square,
                               accum_out=sq_t[:, idx:idx + 1])
                vec.tensor_copy(hi_t[:, idx * 4:idx * 4 + 2], pt[:])
                vec.tensor_sub(hi_t[:, idx * 4 + 2:idx * 4 + 4], pt[:],
                               hi_t[:, idx * 4:idx * 4 + 2])
                act.sqrt(nrm_t[:, idx:idx + 1], ssq_t[:, idx:idx + 1])
                vec.reciprocal(rnrm_t[:, idx:idx + 1], nrm_t[:, idx:idx + 1])
                xt = p1x.tile([128, G], bf, tag="x")
                act.mul(xt[:], gt[:], rnrm_t[:, idx:idx + 1])
                sync.dma_start(xdram, xt[:])

            for jt in range(NT):
                handle_tile(jt, gene[jt * 128:(jt + 1) * 128, :],
                            pos[jt * 128:(jt + 1) * 128, :],
                            ssq, possq, posHL, norm32, rnorm32,
                            xn_dram[jt * 128:(jt + 1) * 128, :])
            for ot in range(OT):
                handle_tile(ot, gown[ot * 128:(ot + 1) * 128, :],
                            posown[ot * 128:(ot + 1) * 128, :],
                            ssqo, nsqo, posHLo, normo, rnormo,
                            xno_dram[ot * 128:(ot + 1) * 128, :])

            vec.tensor_copy(rnb32[:], rnorm32[:])
            # -|pos|^2/2 splits (full j) and own bias
            s32 = p1s.tile([128, NT], f32, tag="s32")
            vec.tensor_scalar_mul(s32[:], possq[:], -0.5)
            vec.tensor_copy(sh32[:], s32[:])
            vec.tensor_sub(sl32[:], s32[:], sh32[:])
            vec.tensor_scalar_mul(nsqo[:], nsqo[:], -0.5)

        for gc in range(GC):
            sync.dma_start_transpose(xnT[:, gc * N:(gc + 1) * N],
                                     xn_dram[:, gc * 128:(gc + 1) * 128])
            sync.dma_start_transpose(xnoT[:, gc * R:(gc + 1) * R],
                                     xno_dram[:, gc * 128:(gc + 1) * 128])

        # broadcast rows: norm_j, 1/norm_j(bf), norm_i(own)
        nb, free_nb = tc.tile([128, N], f32, name="nb")
        with tc.tile_pool(name="rows", bufs=2) as rp:
            _scr_ctr = [0]

            def row_from_cols(dst_row, src_view, scr_pool):
                # src [128, t] -> dram strided (j = t*128+p order) -> row
                n = dst_row.shape[-1]
                i = _scr_ctr[0]
                _scr_ctr[0] += 1
                scr = scr_pool[i % scr_pool.shape[0], 0:n]
                gps.dma_start(scr.rearrange("(t p) -> p t", p=128), src_view)
                gps.dma_start(dst_row, scr[None, :])

            nrow = rp.tile([1, N], f32, tag="nrow")
            row_from_cols(nrow[0:1, :], norm32[:, :], rowscr_f)
            gps.partition_broadcast(nb[:], nrow[:])
            row_from_cols(rn_row[0:1, :], rnb32[:, :], rowscr_b)
            norow = rp.tile([1, R], f32, tag="norow")
            row_from_cols(norow[0:1, :], normo[:, :], rowscr_f)
            gps.partition_broadcast(nbo[:], norow[:])

            # sp rhs rows: [xh, xl, xh, yh, yl, yh, sh, sl]
            pHL = posHL[:].rearrange("p (t f) -> p t f", f=4)
            for r, col in ((0, 0), (1, 2), (2, 0), (3, 1), (4, 3), (5, 1)):
                row_from_cols(sprhs[r:r + 1, :], pHL[:, :, col], rowscr_b)
            row_from_cols(sprhs[6:7, :], sh32[:, :], rowscr_b)
            row_from_cols(sprhs[7:8, :], sl32[:, :], rowscr_b)
            # MLP rhs rows: [1, xh, xh, xl, yh, yh, yl]
            vec.memset(posr7[0:1, :], 1.0)
            for r, col in ((1, 0), (2, 0), (3, 2), (4, 1), (5, 1), (6, 3)):
                row_from_cols(posr7[r:r + 1, :], pHL[:, :, col], rowscr_b)
            pHLo = posHLo[:].rearrange("p (t f) -> p t f", f=4)
            vec.memset(posro7[0:1, :], 1.0)
            for r, col in ((1, 0), (2, 0), (3, 2), (4, 1), (5, 1), (6, 3)):
                row_from_cols(posro7[r:r + 1, :], pHLo[:, :, col], rowscr_b)

        # sp lhsT rows per own tile: [xh, xh, xl, yh, yh, yl, 1, 1]
        with tc.tile_pool(name="sptr", bufs=2) as sp_pool, \
             tc.tile_pool(name="sptr_ps", bufs=2, space="PSUM") as sp_ps8:
            for ot in range(OT):
                t8 = sp_pool.tile([128, 8], bf, tag="t8")
                base = ot * 4
                for c, col in ((0, 0), (1, 0), (2, 2), (3, 1), (4, 1), (5, 3)):
                    vec.tensor_copy(t8[:, c:c + 1],
                                    posHLo[:, base + col:base + col + 1])
                vec.memset(t8[:, 6:8], 1.0)
                ps8 = sp_ps8.tile([8, 128], bf, tag="ps8")
                pe.transpose(ps8[:], t8[:], idb[:])
                vec.tensor_copy(spown[:, ot * 128:(ot + 1) * 128], ps8[:])

        # ---------- P2: K^T, V, Q^T ----------
        with tc.tile_pool(name="bld_ps", bufs=3, space="PSUM") as bps, \
             tc.tile_pool(name="bld_sb", bufs=3) as bsb:
            # K^T chunks [128h, N]: 4 gc matmuls + (bk * 1/norm_j) rank-1,
            # then * norm_j
            for hc in range(HC):
                for jc in range(JC):
                    ps = bps.tile([128, JCW], f32, tag="ps")
                    jsl = slice(jc * JCW, (jc + 1) * JCW)
                    for gc in range(GC):
                        pe.matmul(ps[:],
                                  Wkb[:, gc * H + hc * 128: gc * H + (hc + 1) * 128],
                                  xnT[:, gc * N + jc * JCW: gc * N + (jc + 1) * JCW],
                                  start=(gc == 0), stop=False)
                    pe.matmul(ps[:], bkb[0:1, hc * 128:(hc + 1) * 128],
                              rn_row[0:1, jsl], start=False, stop=True)
                    vec.tensor_tensor(KT[:, hc * N + jc * JCW: hc * N + (jc + 1) * JCW],
                                      ps[:], nb[:, jsl], ALU.mult)
            # V tiles [128j, O]: lhsT = xnT col-block, rhs = Wvb
            for jt in range(NT):
                ps = bps.tile([128, O], f32, tag="psv")
                for gc in range(GC):
                    pe.matmul(ps[:],
                              xnT[:, gc * N + jt * 128: gc * N + (jt + 1) * 128],
                              Wvb[:, gc * O:(gc + 1) * O],
                              start=(gc == 0), stop=(gc == GC - 1))
                vec.scalar_tensor_tensor(Vb[:, jt * O:(jt + 1) * O], ps[:],
                                         norm32[:, jt:jt + 1], bvb[:],
                                         ALU.mult, ALU.add)
            # Q^T chunks [128h, R]
            for hc in range(HC):
                ps = bps.tile([128, R], f32, tag="ps")
                for gc in range(GC):
                    pe.matmul(ps[:],
                              Wqb[:, gc * H + hc * 128: gc * H + (hc + 1) * 128],
                              xnoT[:, gc * R:(gc + 1) * R],
                              start=(gc == 0), stop=(gc == GC - 1))
                qt = bsb.tile([128, R], f32, tag="qt")
                vec.tensor_tensor(qt[:], ps[:], nbo[:], ALU.mult)
                vec.tensor_scalar(QT[:, hc * R:(hc + 1) * R], qt[:],
                                  bq16[:, hc:hc + 1], None, ALU.add)
        free_nb()

        # ---------- P3: MLP (transposed) -> f row, f_own; tanh buffer ----------
        nfb, _nfb_free = tc.tile([128, N], f32, name="nfb")
        _keep.append(_nfb_free)
        with tc.tile_pool(name="mlp_ps", bufs=3, space="PSUM") as mps, \
             tc.tile_pool(name="mlp_fps", bufs=2, space="PSUM") as fps:
            with tc.tile_pool(name="mlp_si", bufs=1) as sip:
                siT = sip.tile([128, HC * N], f16)
                sioT = sip.tile([128, HC * R], f16)
                for hc in range(HC):
                    csl = slice(hc * 128, hc * 128 + 128)
                    for jc in range(JC):
                        ps = mps.tile([128, JCW], f32, tag="ps")
                        pe.matmul(ps[:], WsT7[:, csl],
                                  posr7[:, jc * JCW:(jc + 1) * JCW],
                                  start=True, stop=True)
                        vec.tensor_copy(
                            siT[:, hc * N + jc * JCW: hc * N + (jc + 1) * JCW],
                            ps[:])
                    ps = mps.tile([128, R], f32, tag="ps")
                    pe.matmul(ps[:], WsT7[:, csl], posro7[:],
                              start=True, stop=True)
                    vec.tensor_copy(sioT[:, hc * R:(hc + 1) * R], ps[:])

                with tc.tile_pool(name="mlp_h", bufs=1) as hp:
                    hT = hp.tile([128, HC * N], f16)
                    hoT = hp.tile([128, HC * R], f16)
                    for hc in range(HC):
                        for jc in range(JC):
                            ps = mps.tile([128, JCW], f32, tag="ps")
                            for kc in range(HC):
                                pe.matmul(
                                    ps[:],
                                    W1f[:, kc * H + hc * 128: kc * H + (hc + 1) * 128],
                                    siT[:, kc * N + jc * JCW: kc * N + (jc + 1) * JCW],
                                    start=(kc == 0), stop=(kc == HC - 1))
                            act.activation(
                                hT[:, hc * N + jc * JCW: hc * N + (jc + 1) * JCW],
                                ps[:], AF.Tanh, bias=b1c[:, hc:hc + 1])
                        ps = mps.tile([128, R], f32, tag="ps")
                        for kc in range(HC):
                            pe.matmul(ps[:],
                                      W1f[:, kc * H + hc * 128: kc * H + (hc + 1) * 128],
                                      sioT[:, kc * R:(kc + 1) * R],
                                      start=(kc == 0), stop=(kc == HC - 1))
                        act.activation(hoT[:, hc * R:(hc + 1) * R], ps[:],
                                       AF.Tanh, bias=b1c[:, hc:hc + 1])

                    with tc.tile_pool(name="mlp_e", bufs=1) as ep:
                        eT = ep.tile([128, HC * N], f16)
                        eoT = ep.tile([128, HC * R], f16)
                        for hc in range(HC):
                            for jc in range(JC):
                                ps = mps.tile([128, JCW], f32, tag="ps")
                                for kc in range(HC):
                                    pe.matmul(
                                        ps[:],
                                        W2f[:, kc * H + hc * 128: kc * H + (hc + 1) * 128],
                                        hT[:, kc * N + jc * JCW: kc * N + (jc + 1) * JCW],
                                        start=(kc == 0), stop=(kc == HC - 1))
                                act.activation(
                                    eT[:, hc * N + jc * JCW: hc * N + (jc + 1) * JCW],
                                    ps[:], AF.Relu, bias=b2c[:, hc:hc + 1])
                            ps = mps.tile([128, R], f32, tag="ps")
                            for kc in range(HC):
                                pe.matmul(ps[:],
                                          W2f[:, kc * H + hc * 128: kc * H + (hc + 1) * 128],
                                          hoT[:, kc * R:(kc + 1) * R],
                                          start=(kc == 0), stop=(kc == HC - 1))
                            act.activation(eoT[:, hc * R:(hc + 1) * R], ps[:],
                                           AF.Relu, bias=b2c[:, hc:hc + 1])

                        # f row (full): negf = -(e @ Wf + bf); broadcast
                        with tc.tile_pool(name="frow", bufs=1) as fp:
                            nfrow = fp.tile([1, N], f32)
                            negbf = fp.tile([1, 1], f32)
                            vec.tensor_scalar_mul(negbf[:], bfbc[0:1, :], -1.0)
                            for jc in range(JC):
                                ps = fps.tile([1, JCW], f32, tag="psf")
                                for kc in range(HC):
                                    pe.matmul(ps[:], Wff[:, kc:kc + 1],
                                              eT[:, kc * N + jc * JCW: kc * N + (jc + 1) * JCW],
                                              start=(kc == 0), stop=(kc == HC - 1))
                                act.activation(nfrow[0:1, jc * JCW:(jc + 1) * JCW],
                                               ps[:], AF.Identity,
                                               bias=negbf[0:1, :], scale=-1.0)
                            gps.partition_broadcast(nfb[:], nfrow[:])
                            # f_own per tile: [128,1] psums
                            for ot in range(OT):
                                ps = fps.tile([128, 1], f32, tag="psf")
                                for kc in range(HC):
                                    pe.matmul(
                                        ps[:],
                                        eoT[:, kc * R + ot * 128: kc * R + (ot + 1) * 128],
                                        Wff[:, kc:kc + 1],
                                        start=(kc == 0), stop=(kc == HC - 1))
                                act.activation(fown[:, ot:ot + 1], ps[:],
                                               AF.Identity, bias=bfbc[:, 0:1])

        # tanh(f_i - f_j) buffer (bf16), per own tile
        tanhb = T([128, OT * N], bf, name="tanhb")    # 32K/p
        for ot in range(OT):
            for jc in range(JC):
                jsl = slice(jc * JCW, (jc + 1) * JCW)
                act.activation(tanhb[:, ot * N + jc * JCW: ot * N + (jc + 1) * JCW],
                               nfb[:, jsl], AF.Tanh, bias=fown[:, ot:ot + 1])

        # ---------- P4: main loop ----------
        with tc.tile_pool(name="sp_ps", bufs=2, space="PSUM") as sp_ps, \
             tc.tile_pool(name="mn_ps", bufs=2, space="PSUM") as mn_ps, \
             tc.tile_pool(name="tr_ps", bufs=2, space="PSUM") as tr_ps, \
             tc.tile_pool(name="u_ps", bufs=2, space="PSUM") as u_ps, \
             tc.tile_pool(name="spex_p", bufs=3) as spex_p, \
             tc.tile_pool(name="numer_p", bufs=2) as numer_p, \
             tc.tile_pool(name="rs_p", bufs=2) as rs_p, \
             tc.tile_pool(name="attn_p", bufs=4) as attn_p, \
             tc.tile_pool(name="trs_p", bufs=3) as trs_p, \
             tc.tile_pool(name="upd_p", bufs=2) as upd_p:
            for ot in range(OT):
                osl = slice(ot * 128, (ot + 1) * 128)
                numer = numer_p.tile([128, N], bf, tag="numer")
                rs = rs_p.tile([128, JC], f32, tag="rs")
                for jc in range(JC):
                    jsl = slice(jc * JCW, (jc + 1) * JCW)
                    psA = sp_ps.tile([128, JCW], f32, tag="psA")
                    pe.matmul(psA[:], spown[:, osl], sprhs[:, jsl],
                              start=True, stop=True)
                    spex = spex_p.tile([128, JCW], bf, tag="spex")
                    act.activation(spex[:], psA[:], AF.Exp,
                                   bias=nsqo[:, ot:ot + 1])
                    psM = mn_ps.tile([128, JCW], f32, tag="psM")
                    for gc in range(GC):
                        pe.matmul(psM[:],
                                  xnoT[:, gc * R + ot * 128: gc * R + (ot + 1) * 128],
                                  xnT[:, gc * N + jc * JCW: gc * N + (jc + 1) * JCW],
                                  start=(gc == 0), stop=False)
                    for hc in range(HC):
                        pe.matmul(psM[:],
                                  QT[:, hc * R + ot * 128: hc * R + (ot + 1) * 128],
                                  KT[:, hc * N + jc * JCW: hc * N + (jc + 1) * JCW],
                                  start=False, stop=False)
                    pe.matmul(psM[:], idb[:],
                              tanhb[:, ot * N + jc * JCW: ot * N + (jc + 1) * JCW],
                              start=False, stop=False)
                    pe.matmul(psM[:], idb[:], spex[:], start=False, stop=True)
                    act.activation(numer[:, jsl], psM[:], AF.Exp,
                                   accum_out=rs[:, jc:jc + 1])
                rsum = rs_p.tile([128, 1], f32, tag="rsum")
                vec.reduce_sum(rsum[:], rs[:], axis=mybir.AxisListType.X)
                recip = rs_p.tile([128, 1], f32, tag="recip")
                vec.reciprocal(recip[:], rsum[:])
                for jc in range(JC):
                    jsl = slice(jc * JCW, (jc + 1) * JCW)
                    atile = attn_p.tile([128, JCW], f32, tag="atile")
                    vec.tensor_scalar_mul(atile[:], numer[:, jsl], recip[:])
                    sync.dma_start(attn_out[osl, jsl], atile[:])
                # attn @ V with PE-transposed numer blocks
                psU = u_ps.tile([128, O], f32, tag="psU")
                for jc in range(JC):
                    pstr = tr_ps.tile([128, JCW], bf, tag="pstr")
                    for k in range(4):
                        jb = jc * 4 + k
                        pe.transpose(pstr[:, k * 128:(k + 1) * 128],
                                     numer[:, jb * 128:(jb + 1) * 128], idb[:])
                    trs = trs_p.tile([128, JCW], bf, tag="trs")
                    vec.tensor_copy(trs[:], pstr[:])
                    for k in range(4):
                        jb = jc * 4 + k
                        pe.matmul(psU[:], trs[:, k * 128:(k + 1) * 128],
                                  Vb[:, jb * O:(jb + 1) * O],
                                  start=(jb == 0), stop=(jb == NT - 1))
                utile = upd_p.tile([128, O], f32, tag="utile")
                vec.tensor_scalar_mul(utile[:], psU[:], recip[:])
                sync.dma_start(upd_out[osl, :], utile[:])

        # release single-tile pools (LIFO) before the context schedules;
        # a GC-time release would inject boundary insts into the final BIR
        for free in reversed(_keep):
            free()

    nc.compile()
    return nc


_NC_CACHE = {}


def _get_nc():
    if "nc" not in _NC_CACHE:
        _NC_CACHE["nc"] = _build_nc()
    return _NC_CACHE["nc"]


def kernel(**inputs):
    from concourse.bass_utils import run_bass_kernel_spmd

    inp = {k: np.asarray(v, dtype=np.float32) for k, v in inputs.items()}
    nc = _get_nc()

    shared = {k: np.ascontiguousarray(inp[k]) for k in
              ("gene_expr", "spatial_pos", "Wq", "bq", "Wk", "bk", "Wv", "bv",
               "Ws", "bs", "W1", "b1", "W2", "b2", "Wf", "bf")}
    shared["ident"] = np.eye(128, dtype=ml_dtypes.bfloat16)

    in_maps = []
    for c in range(NCORES):
        m = dict(shared)
        m["gene_own"] = np.ascontiguousarray(inp["gene_expr"][c * R:(c + 1) * R])
        m["pos_own"] = np.ascontiguousarray(inp["spatial_pos"][c * R:(c + 1) * R])
        in_maps.append(m)

    res = run_bass_kernel_spmd(nc, in_maps, core_ids=list(range(NCORES)),
                               trace=bool(int(os.environ.get("KERNEL_TRACE", "0"))))
    _NC_CACHE["last_result"] = res
    attn = np.concatenate([r["attn_own"] for r in res.results], axis=0)
    upd = np.concatenate([r["upd_own"] for r in res.results], axis=0)
    return attn, upd


# revision 20
# speedup vs baseline: 9364.5968x; 9364.5968x over previous
# Trainium2 Bass kernel for nn_CrossAttentionSpatialWeight.
#
# Row-sharded across 8 NeuronCores: core c owns query rows [c*512, (c+1)*512).
# Each core computes (redundantly) the full K^T / V / spatial-MLP features from
# the full inputs, then its 512x4096 block of logits
#   logits = Q K^T/16 + exp(-d2/2) + cos_sim + tanh(f_i - f_j)
# accumulated in PSUM (es + qk via bf16 matmuls; rank-1/2 terms via identity
# matmuls of ACT-produced tiles), then softmax (no max-subtract needed: |logits|
# <~ 6) and attn @ V via PE-transposed numerator blocks.
#
# The gene path is pipelined in 4 column-quarters (1024 j each): each quarter's
# norms/xn/transpose/K^T/V only depend on that quarter's tiles, so builds and
# the j-stream overlap. ACT table switches are limited to tanh -> sqrt -> exp.
import os
import numpy as np
import ml_dtypes

N, G, H, O = 4096, 512, 256, 256
NCORES = 8
R = N // NCORES          # 512 own rows per core
OT = R // 128            # 4 own-row tiles
NT = N // 128            # 32 j-tiles
GC = G // 128            # 4 gene-dim chunks
HC = H // 128            # 2 hidden chunks
JC = 8                   # j-chunks per row-tile
JCW = N // JC            # 512
NQ = 4                   # column quarters
QR = N // NQ             # 1024 j per quarter
TPQ = QR // 128          # 8 tiles per quarter
CPQ = QR // JCW          # 2 jc-chunks per quarter


def _build_nc():
    import concourse.bacc as bacc
    import concourse.mybir as mybir
    import concourse.tile as tile

    dt = mybir.dt
    f32, bf, f16 = dt.float32, dt.bfloat16, dt.float16
    AF = mybir.ActivationFunctionType
    ALU = mybir.AluOpType

    nc = bacc.Bacc("TRN2", target_bir_lowering=False, debug=False,
                   num_devices=NCORES)

    def din(name, shape):
        return nc.dram_tensor(name, list(shape), f32, kind="ExternalInput").ap()

    gene = din("gene_expr", (N, G))
    pos = din("spatial_pos", (N, 2))
    gown = din("gene_own", (R, G))
    posown = din("pos_own", (R, 2))
    Wq, bq = din("Wq", (G, H)), din("bq", (H,))
    Wk, bk = din("Wk", (G, H)), din("bk", (H,))
    Wv, bv = din("Wv", (G, O)), din("bv", (O,))
    Ws, bs = din("Ws", (2, H)), din("bs", (H,))
    W1, b1 = din("W1", (H, H)), din("b1", (H,))
    W2, b2 = din("W2", (H, H)), din("b2", (H,))
    Wf, bf_b = din("Wf", (H, 1)), din("bf", (1,))
    ident = nc.dram_tensor("ident", [128, 128], bf, kind="ExternalInput").ap()

    attn_out = nc.dram_tensor("attn_own", [R, N], f32, kind="ExternalOutput").ap()
    upd_out = nc.dram_tensor("upd_own", [R, O], f32, kind="ExternalOutput").ap()

    xn_dram = [nc.dram_tensor(f"xn_scratch{q}", [QR, G], bf, kind="Internal").ap()
               for q in range(NQ)]
    xno_dram = nc.dram_tensor("xno_scratch", [R, G], bf, kind="Internal").ap()
    rowscr_f = nc.dram_tensor("rowscr_f", [12, N], f32, kind="Internal").ap()
    rowscr_b = nc.dram_tensor("rowscr_b", [24, N], bf, kind="Internal").ap()

    with tile.TileContext(nc) as tc:
        act, vec, pe, sync, gps = nc.scalar, nc.vector, nc.tensor, nc.sync, nc.gpsimd

        _keep = []  # keep free-handles alive so GC can't close single pools

        def T(shape, dtype, name):
            t, free = tc.tile(shape, dtype, name=name)
            _keep.append(free)
            return t

        # ---------- persistent SBUF ----------
        idb = T([128, 128], bf, name="idb")
        sync.dma_start(idb[:], ident)

        # per-quarter big matrices
        xnTq = [T([128, GC * QR], bf, name=f"xnTq{q}") for q in range(NQ)]
        KTq = [T([128, HC * QR], bf, name=f"KTq{q}") for q in range(NQ)]
        Vq = [T([128, TPQ * O], bf, name=f"Vq{q}") for q in range(NQ)]
        nbq = [T([128, QR], f32, name=f"nbq{q}") for q in range(NQ)]
        rnrowq = [T([1, QR], bf, name=f"rnrowq{q}") for q in range(NQ)]
        ssqq = [T([128, TPQ], f32, name=f"ssqq{q}") for q in range(NQ)]
        normq = [T([128, TPQ], f32, name=f"normq{q}") for q in range(NQ)]
        rnormq = [T([128, TPQ], f32, name=f"rnormq{q}") for q in range(NQ)]
        rnbq = [T([128, TPQ], bf, name=f"rnbq{q}") for q in range(NQ)]

        xnoT = T([128, GC * R], bf, name="xnoT")      # own xn^T (lhsT)
        QT = T([128, HC * R], bf, name="QT")          # own Q^T/16
        spown = T([8, OT * 128], bf, name="spown")    # sp lhsT rows
        sprhs = T([8, N], bf, name="sprhs")           # sp rhs rows
        posr7 = T([7, N], bf, name="posr7")           # MLP si rhs rows
        posro7 = T([7, R], bf, name="posro7")
        WsT7 = T([7, HC * 128], bf, name="WsT7")      # MLP si lhsT rows
        fown = T([128, OT], f32, name="fown")
        nsqo = T([128, OT], f32, name="nsqo")         # -|pos_i|^2/2
        normo = T([128, OT], f32, name="normo")
        rnormo = T([128, OT], f32, name="rnormo")
        nbo = T([128, R], f32, name="nbo")            # norm_i bcast (own)
        bfbc = T([128, 1], f32, name="bfbc")
        ssqo = T([128, OT], f32, name="ssqo")
        possq = T([128, NT], f32, name="possq")
        posHL = T([128, NT * 4], bf, name="posHL")    # [xh yh xl yl]/jt
        posHLo = T([128, OT * 4], bf, name="posHLo")
        sh32 = T([128, NT], bf, name="sh32")          # hi(-|p_j|^2/2)
        sl32 = T([128, NT], bf, name="sl32")          # lo residual

        # weights (bf16/f16 casts)
        Wqb = T([128, GC * H], bf, name="Wqb")
        Wkb = T([128, GC * H], bf, name="Wkb")
        Wvb = T([128, GC * O], bf, name="Wvb")
        W1f = T([128, HC * H], f16, name="W1f")
        W2f = T([128, HC * H], f16, name="W2f")
        Wff = T([128, HC], f16, name="Wff")
        bq16 = T([128, HC], f32, name="bq16")
        bkb = T([1, H], bf, name="bkb")
        b1c = T([128, HC], f32, name="b1c")
        b2c = T([128, HC], f32, name="b2c")
        bvb = T([128, O], f32, name="bvb")

        with tc.tile_pool(name="wload", bufs=2) as wl:
            for W, Wc, n_out, scale in ((Wq, Wqb, H, 1.0 / 16.0),
                                        (Wk, Wkb, H, 1.0), (Wv, Wvb, O, 1.0)):
                t = wl.tile([128, GC * n_out], f32, tag="w")
                for gc in range(GC):
                    sync.dma_start(t[:, gc * n_out:(gc + 1) * n_out],
                                   W[gc * 128:(gc + 1) * 128, :])
                act.mul(Wc[:], t[:], scale)
            for W, Wc in ((W1, W1f), (W2, W2f)):
                t = wl.tile([128, HC * H], f32, tag="w")
                for kc in range(HC):
                    sync.dma_start(t[:, kc * H:(kc + 1) * H],
                                   W[kc * 128:(kc + 1) * 128, :])
                act.mul(Wc[:], t[:], 1.0)
            t = wl.tile([128, HC], f32, tag="small")
            sync.dma_start(t[:], Wf.rearrange("(c p) o -> p (c o)", p=128))
            act.mul(Wff[:], t[:], 1.0)
            # biases
            t = wl.tile([128, HC], f32, tag="small")
            sync.dma_start(t[:], bq.rearrange("(c p) -> p c", p=128))
            vec.tensor_scalar_mul(bq16[:], t[:], 1.0 / 16.0)
            sync.dma_start(b1c[:], b1.rearrange("(c p) -> p c", p=128))
            sync.dma_start(b2c[:], b2.rearrange("(c p) -> p c", p=128))
            t2 = wl.tile([1, H], f32, tag="row")
            sync.dma_start(t2[:], bk[None, :])
            vec.tensor_copy(bkb[:], t2[:])
            t3 = wl.tile([1, O], f32, tag="row")
            sync.dma_start(t3[:], bv[None, :])
            gps.partition_broadcast(bvb[:], t3[:])
            t4 = wl.tile([1, 1], f32, tag="tiny")
            sync.dma_start(t4[:], bf_b[None, :])
            gps.partition_broadcast(bfbc[:], t4[:])
            # WsT7 rows = [bs, Wxh, Wxl, Wxh, Wyh, Wyl, Wyh]
            # paired with posr7 rows [1, xh, xh, xl, yh, yh, yl]
            tws = wl.tile([2, H], f32, tag="row2")
            sync.dma_start(tws[:], Ws[:, :])
            tbs = wl.tile([1, H], f32, tag="row")
            sync.dma_start(tbs[:], bs[None, :])
            whl = wl.tile([2, H], bf, tag="row2b")   # hi
            wlo = wl.tile([2, H], bf, tag="row2c")   # lo
            vec.tensor_copy(whl[:], tws[:])
            vec.tensor_sub(wlo[:], tws[:], whl[:])
            tbsb = wl.tile([1, H], bf, tag="rowb")
            vec.tensor_copy(tbsb[:], tbs[:])
            for r, srcrow in ((0, tbsb[0:1, :]), (1, whl[0:1, :]),
                              (2, wlo[0:1, :]), (3, whl[0:1, :]),
                              (4, whl[1:2, :]), (5, wlo[1:2, :]),
                              (6, whl[1:2, :])):
                gps.dma_start(WsT7[r:r + 1, :], srcrow)

        # ---------- own-row gene prep (feeds Q^T early) ----------
        with tc.tile_pool(name="ownp", bufs=5) as op:
            own_gt = []
            for ot in range(OT):
                gt = op.tile([128, G], f32, tag="g")
                sync.dma_start(gt[:], gown[ot * 128:(ot + 1) * 128, :])
                own_gt.append(gt)
                scr = op.tile([128, G], bf, tag="scr")
                act.activation(scr[:], gt[:], AF.Square,
                               accum_out=ssqo[:, ot:ot + 1])
            act.sqrt(normo[:], ssqo[:])
            vec.reciprocal(rnormo[:], normo[:])
            for ot in range(OT):
                xt = op.tile([128, G], bf, tag="x")
                vec.tensor_scalar_mul(xt[:], own_gt[ot][:],
                                      rnormo[:, ot:ot + 1])
                gps.dma_start(xno_dram[ot * 128:(ot + 1) * 128, :], xt[:])
            for gc in range(GC):
                sync.dma_start_transpose(xnoT[:, gc * R:(gc + 1) * R],
                                         xno_dram[:, gc * 128:(gc + 1) * 128])
            norow = op.tile([1, R], f32, tag="norow")
            sync.dma_start(rowscr_f[8, 0:R].rearrange("(t p) -> p t", p=128),
                           normo[:, :])
            sync.dma_start(norow[0:1, :], rowscr_f[8, 0:R][None, :])
            gps.partition_broadcast(nbo[:], norow[:])

        # ---------- pos tiles: squares, hi/lo splits ----------
        with tc.tile_pool(name="posp", bufs=4) as pp:
            def pos_tile(idx, src_p, sq_t, hi_t):
                pt = pp.tile([128, 2], f32, tag="pos")
                sync.dma_start(pt[:], src_p)
                pscr = pp.tile([128, 2], f32, tag="pscr")
                act.activation(pscr[:], pt[:], AF.Square,
                               accum_out=sq_t[:, idx:idx + 1])
                vec.tensor_copy(hi_t[:, idx * 4:idx * 4 + 2], pt[:])
                vec.tensor_sub(hi_t[:, idx * 4 + 2:idx * 4 + 4], pt[:],
                               hi_t[:, idx * 4:idx * 4 + 2])

            for jt in range(NT):
                pos_tile(jt, pos[jt * 128:(jt + 1) * 128, :], possq, posHL)
            for ot in range(OT):
                pos_tile(ot, posown[ot * 128:(ot + 1) * 128, :], nsqo, posHLo)

            s32 = pp.tile([128, NT], f32, tag="s32")
            vec.tensor_scalar_mul(s32[:], possq[:], -0.5)
            vec.tensor_copy(sh32[:], s32[:])
            vec.tensor_sub(sl32[:], s32[:], sh32[:])
            vec.tensor_scalar_mul(nsqo[:], nsqo[:], -0.5)

        # pos-derived rows (DMA gathers via DRAM scratch)
        with tc.tile_pool(name="rows", bufs=2) as rp:
            _scr_ctr = [0]

            def row_from_cols(dst_row, src_view, scr_pool):
                # src [128, t] -> dram strided (j = t*128+p order) -> row
                n = dst_row.shape[-1]
                i = _scr_ctr[0]
                _scr_ctr[0] += 1
                scr = scr_pool[i % scr_pool.shape[0], 0:n]
                sync.dma_start(scr.rearrange("(t p) -> p t", p=128), src_view)
                sync.dma_start(dst_row, scr[None, :])

            # sp rhs rows: [xh, xl, xh, yh, yl, yh, sh, sl]
            pHL = posHL[:].rearrange("p (t f) -> p t f", f=4)
            for r, col in ((0, 0), (1, 2), (2, 0), (3, 1), (4, 3), (5, 1)):
                row_from_cols(sprhs[r:r + 1, :], pHL[:, :, col], rowscr_b)
            row_from_cols(sprhs[6:7, :], sh32[:, :], rowscr_b)
            row_from_cols(sprhs[7:8, :], sl32[:, :], rowscr_b)
            # MLP rhs rows: [1, xh, xh, xl, yh, yh, yl]
            vec.memset(posr7[0:1, :], 1.0)
            for r, col in ((1, 0), (2, 0), (3, 2), (4, 1), (5, 1), (6, 3)):
                row_from_cols(posr7[r:r + 1, :], pHL[:, :, col], rowscr_b)
            pHLo = posHLo[:].rearrange("p (t f) -> p t f", f=4)
            vec.memset(posro7[0:1, :], 1.0)
            for r, col in ((1, 0), (2, 0), (3, 2), (4, 1), (5, 1), (6, 3)):
                row_from_cols(posro7[r:r + 1, :], pHLo[:, :, col], rowscr_b)

        # sp lhsT rows per own tile: [xh, xh, xl, yh, yh, yl, 1, 1]
        with tc.tile_pool(name="sptr", bufs=2) as sp_pool, \
             tc.tile_pool(name="sptr_ps", bufs=2, space="PSUM") as sp_ps8:
            for ot in range(OT):
                t8 = sp_pool.tile([128, 8], bf, tag="t8")
                base = ot * 4
                for c, col in ((0, 0), (1, 0), (2, 2), (3, 1), (4, 1), (5, 3)):
                    vec.tensor_copy(t8[:, c:c + 1],
                                    posHLo[:, base + col:base + col + 1])
                vec.memset(t8[:, 6:8], 1.0)
                ps8 = sp_ps8.tile([8, 128], bf, tag="ps8")
                pe.transpose(ps8[:], t8[:], idb[:])
                vec.tensor_copy(spown[:, ot * 128:(ot + 1) * 128], ps8[:])

        # ---------- MLP (transposed) -> f row, f_own; tanh buffer ----------
        tanhb = T([128, OT * N], bf, name="tanhb")  # tanh(f_i-f_j)
        with tc.tile_pool(name="mlp_ps", bufs=3, space="PSUM") as mps, \
             tc.tile_pool(name="mlp_fps", bufs=2, space="PSUM") as fps, \
             tc.tile_pool(name="mlp_sb", bufs=2) as mlp_sb, \
             tc.tile_pool(name="frow_sb", bufs=3) as frow_sb:
            # layer buffers rotate through 2 slots (si -> h -> e)
            siT = mlp_sb.tile([128, HC * N], f16, tag="layer")
            sioT = mlp_sb.tile([128, HC * R], f16, tag="layero")
            for hc in range(HC):
                csl = slice(hc * 128, hc * 128 + 128)
                for jc in range(JC):
                    ps = mps.tile([128, JCW], f32, tag="ps")
                    pe.matmul(ps[:], WsT7[:, csl],
                              posr7[:, jc * JCW:(jc + 1) * JCW],
                              start=True, stop=True)
                    vec.tensor_copy(
                        siT[:, hc * N + jc * JCW: hc * N + (jc + 1) * JCW],
                        ps[:])
                ps = mps.tile([128, R], f32, tag="ps")
                pe.matmul(ps[:], WsT7[:, csl], posro7[:],
                          start=True, stop=True)
                vec.tensor_copy(sioT[:, hc * R:(hc + 1) * R], ps[:])

            hT = mlp_sb.tile([128, HC * N], f16, tag="layer")
            hoT = mlp_sb.tile([128, HC * R], f16, tag="layero")
            for hc in range(HC):
                for jc in range(JC):
                    ps = mps.tile([128, JCW], f32, tag="ps")
                    for kc in range(HC):
                        pe.matmul(
                            ps[:],
                            W1f[:, kc * H + hc * 128: kc * H + (hc + 1) * 128],
                            siT[:, kc * N + jc * JCW: kc * N + (jc + 1) * JCW],
                            start=(kc == 0), stop=(kc == HC - 1))
                    act.activation(
                        hT[:, hc * N + jc * JCW: hc * N + (jc + 1) * JCW],
                        ps[:], AF.Tanh, bias=b1c[:, hc:hc + 1])
                ps = mps.tile([128, R], f32, tag="ps")
                for kc in range(HC):
                    pe.matmul(ps[:],
                              W1f[:, kc * H + hc * 128: kc * H + (hc + 1) * 128],
                              sioT[:, kc * R:(kc + 1) * R],
                              start=(kc == 0), stop=(kc == HC - 1))
                act.activation(hoT[:, hc * R:(hc + 1) * R], ps[:],
                               AF.Tanh, bias=b1c[:, hc:hc + 1])

            eT = mlp_sb.tile([128, HC * N], f16, tag="layer")
            eoT = mlp_sb.tile([128, HC * R], f16, tag="layero")
            for hc in range(HC):
                for jc in range(JC):
                    ps = mps.tile([128, JCW], f32, tag="ps")
                    for kc in range(HC):
                        pe.matmul(
                            ps[:],
                            W2f[:, kc * H + hc * 128: kc * H + (hc + 1) * 128],
                            hT[:, kc * N + jc * JCW: kc * N + (jc + 1) * JCW],
                            start=(kc == 0), stop=(kc == HC - 1))
                    act.activation(
                        eT[:, hc * N + jc * JCW: hc * N + (jc + 1) * JCW],
                        ps[:], AF.Relu, bias=b2c[:, hc:hc + 1])
                ps = mps.tile([128, R], f32, tag="ps")
                for kc in range(HC):
                    pe.matmul(ps[:],
                              W2f[:, kc * H + hc * 128: kc * H + (hc + 1) * 128],
                              hoT[:, kc * R:(kc + 1) * R],
                              start=(kc == 0), stop=(kc == HC - 1))
                act.activation(eoT[:, hc * R:(hc + 1) * R], ps[:],
                               AF.Relu, bias=b2c[:, hc:hc + 1])

            # f_own per tile (tanh bias), then per-jc f rows -> tanh buffer
            negbf = frow_sb.tile([1, 1], f32, bufs=1)
            vec.tensor_scalar_mul(negbf[:], bfbc[0:1, :], -1.0)
            for ot in range(OT):
                ps = fps.tile([128, 1], f32, tag="psf")
                for kc in range(HC):
                    pe.matmul(
                        ps[:],
                        eoT[:, kc * R + ot * 128: kc * R + (ot + 1) * 128],
                        Wff[:, kc:kc + 1],
                        start=(kc == 0), stop=(kc == HC - 1))
                act.activation(fown[:, ot:ot + 1], ps[:],
                               AF.Identity, bias=bfbc[:, 0:1])
            for jc in range(JC):
                ps = fps.tile([1, JCW], f32, tag="psf")
                for kc in range(HC):
                    pe.matmul(ps[:], Wff[:, kc:kc + 1],
                              eT[:, kc * N + jc * JCW: kc * N + (jc + 1) * JCW],
                              start=(kc == 0), stop=(kc == HC - 1))
                nfrow = frow_sb.tile([1, JCW], f32, tag="nfrow")
                act.activation(nfrow[0:1, :], ps[:], AF.Identity,
                               bias=negbf[0:1, :], scale=-1.0)
                nfc = frow_sb.tile([128, JCW], f32, tag="nfc")
                gps.partition_broadcast(nfc[:], nfrow[:])
                for ot in range(OT):
                    act.activation(
                        tanhb[:, ot * N + jc * JCW: ot * N + (jc + 1) * JCW],
                        nfc[:], AF.Tanh, bias=fown[:, ot:ot + 1])

        # ---------- Q^T (own) ----------
        with tc.tile_pool(name="q_ps", bufs=2, space="PSUM") as qps, \
             tc.tile_pool(name="q_sb", bufs=2) as qsb:
            for hc in range(HC):
                ps = qps.tile([128, R], f32, tag="ps")
                for gc in range(GC):
                    pe.matmul(ps[:],
                              Wqb[:, gc * H + hc * 128: gc * H + (hc + 1) * 128],
                              xnoT[:, gc * R:(gc + 1) * R],
                              start=(gc == 0), stop=(gc == GC - 1))
                qt = qsb.tile([128, R], f32, tag="qt")
                vec.tensor_tensor(qt[:], ps[:], nbo[:], ALU.mult)
                vec.tensor_scalar(QT[:, hc * R:(hc + 1) * R], qt[:],
                                  bq16[:, hc:hc + 1], None, ALU.add)

        # ---------- gene stream + K^T/V builds, pipelined per quarter ------
        with tc.tile_pool(name="p1g", bufs=9) as p1g, \
             tc.tile_pool(name="p1x", bufs=3) as p1x, \
             tc.tile_pool(name="p1s", bufs=3) as p1s, \
             tc.tile_pool(name="nrows", bufs=1) as nrp, \
             tc.tile_pool(name="bld_ps", bufs=3, space="PSUM") as bps:
            for q in range(NQ):
                qbase = q * TPQ
                # squares + group norms + cast from retained tiles (no reload)
                for grp in range(TPQ // 4):
                    gsl = slice(grp * 4, (grp + 1) * 4)
                    gts = []
                    for i in range(grp * 4, (grp + 1) * 4):
                        jt = qbase + i
                        gt = p1g.tile([128, G], f32, tag="g")
                        sync.dma_start(gt[:], gene[jt * 128:(jt + 1) * 128, :])
                        gts.append(gt)
                        scr = p1s.tile([128, G], bf, tag="scr")
                        act.activation(scr[:], gt[:], AF.Square,
                                       accum_out=ssqq[q][:, i:i + 1])
                    act.sqrt(normq[q][:, gsl], ssqq[q][:, gsl])
                    vec.reciprocal(rnormq[q][:, gsl], normq[q][:, gsl])
                    for n_, i in enumerate(range(grp * 4, (grp + 1) * 4)):
                        xt = p1x.tile([128, G], bf, tag="x")
                        vec.tensor_scalar_mul(xt[:], gts[n_][:],
                                              rnormq[q][:, i:i + 1])
                        gps.dma_start(xn_dram[q][i * 128:(i + 1) * 128, :],
                                       xt[:])
                vec.tensor_copy(rnbq[q][:], rnormq[q][:])
                for gc in range(GC):
                    sync.dma_start_transpose(
                        xnTq[q][:, gc * QR:(gc + 1) * QR],
                        xn_dram[q][:, gc * 128:(gc + 1) * 128])
                # norm rows for this quarter
                nrow = nrp.tile([1, QR], f32, tag="nrow")
                sync.dma_start(rowscr_f[q, 0:QR].rearrange("(t p) -> p t", p=128),
                               normq[q][:, :])
                sync.dma_start(nrow[0:1, :], rowscr_f[q, 0:QR][None, :])
                gps.partition_broadcast(nbq[q][:], nrow[:])
                sync.dma_start(rowscr_b[16 + q, 0:QR].rearrange("(t p) -> p t", p=128),
                               rnbq[q][:, :])
                sync.dma_start(rnrowq[q][0:1, :], rowscr_b[16 + q, 0:QR][None, :])
                # K^T build for this quarter
                for hc in range(HC):
                    for c in range(CPQ):
                        ps = bps.tile([128, JCW], f32, tag="ps")
                        csl = slice(c * JCW, (c + 1) * JCW)
                        for gc in range(GC):
                            pe.matmul(
                                ps[:],
                                Wkb[:, gc * H + hc * 128: gc * H + (hc + 1) * 128],
                                xnTq[q][:, gc * QR + c * JCW: gc * QR + (c + 1) * JCW],
                                start=(gc == 0), stop=False)
                        pe.matmul(ps[:], bkb[0:1, hc * 128:(hc + 1) * 128],
                                  rnrowq[q][0:1, csl], start=False, stop=True)
                        vec.tensor_tensor(
                            KTq[q][:, hc * QR + c * JCW: hc * QR + (c + 1) * JCW],
                            ps[:], nbq[q][:, csl], ALU.mult)
                # V build for this quarter
                for i in range(TPQ):
                    ps = bps.tile([128, O], f32, tag="psv")
                    for gc in range(GC):
                        pe.matmul(ps[:],
                                  xnTq[q][:, gc * QR + i * 128: gc * QR + (i + 1) * 128],
                                  Wvb[:, gc * O:(gc + 1) * O],
                                  start=(gc == 0), stop=(gc == GC - 1))
                    vec.scalar_tensor_tensor(Vq[q][:, i * O:(i + 1) * O], ps[:],
                                             normq[q][:, i:i + 1], bvb[:],
                                             ALU.mult, ALU.add)

        # ---------- main loop ----------
        with tc.tile_pool(name="sp_ps", bufs=2, space="PSUM") as sp_ps, \
             tc.tile_pool(name="mn_ps", bufs=2, space="PSUM") as mn_ps, \
             tc.tile_pool(name="tr_ps", bufs=2, space="PSUM") as tr_ps, \
             tc.tile_pool(name="u_ps", bufs=2, space="PSUM") as u_ps, \
             tc.tile_pool(name="spex_p", bufs=3) as spex_p, \
             tc.tile_pool(name="numer_p", bufs=2) as numer_p, \
             tc.tile_pool(name="rs_p", bufs=2) as rs_p, \
             tc.tile_pool(name="attn_p", bufs=2) as attn_p, \
             tc.tile_pool(name="trs_p", bufs=3) as trs_p, \
             tc.tile_pool(name="upd_p", bufs=2) as upd_p:
            for ot in range(OT):
                osl = slice(ot * 128, (ot + 1) * 128)
                numer = numer_p.tile([128, N], bf, tag="numer")
                rs = rs_p.tile([128, JC], f32, tag="rs")
                for jc in range(JC):
                    q, c = jc // CPQ, jc % CPQ
                    jsl = slice(jc * JCW, (jc + 1) * JCW)
                    psA = sp_ps.tile([128, JCW], f32, tag="psA")
                    pe.matmul(psA[:], spown[:, osl], sprhs[:, jsl],
                              start=True, stop=True)
                    spex = spex_p.tile([128, JCW], bf, tag="spex")
                    act.activation(spex[:], psA[:], AF.Exp,
                                   bias=nsqo[:, ot:ot + 1])
                    psM = mn_ps.tile([128, JCW], f32, tag="psM")
                    for gc in range(GC):
                        pe.matmul(psM[:],
                                  xnoT[:, gc * R + ot * 128: gc * R + (ot + 1) * 128],
                                  xnTq[q][:, gc * QR + c * JCW: gc * QR + (c + 1) * JCW],
                                  start=(gc == 0), stop=False)
                    for hc in range(HC):
                        pe.matmul(psM[:],
                                  QT[:, hc * R + ot * 128: hc * R + (ot + 1) * 128],
                                  KTq[q][:, hc * QR + c * JCW: hc * QR + (c + 1) * JCW],
                                  start=False, stop=False)
                    pe.matmul(psM[:], idb[:],
                              tanhb[:, ot * N + jc * JCW: ot * N + (jc + 1) * JCW],
                              start=False, stop=False)
                    pe.matmul(psM[:], idb[:], spex[:], start=False, stop=True)
                    act.activation(numer[:, jsl], psM[:], AF.Exp,
                                   accum_out=rs[:, jc:jc + 1])
                rsum = rs_p.tile([128, 1], f32, tag="rsum")
                vec.reduce_sum(rsum[:], rs[:], axis=mybir.AxisListType.X)
                recip = rs_p.tile([128, 1], f32, tag="recip")
                vec.reciprocal(recip[:], rsum[:])
                for jc in range(JC):
                    jsl = slice(jc * JCW, (jc + 1) * JCW)
                    atile = attn_p.tile([128, JCW], f32, tag="atile")
                    vec.tensor_scalar_mul(atile[:], numer[:, jsl], recip[:])
                    sync.dma_start(attn_out[osl, jsl], atile[:])
                # attn @ V with PE-transposed numer blocks
                psU = u_ps.tile([128, O], f32, tag="psU")
                for jc in range(JC):
                    q = jc // CPQ
                    pstr = tr_ps.tile([128, JCW], bf, tag="pstr")
                    for k in range(4):
                        jb = jc * 4 + k
                        pe.transpose(pstr[:, k * 128:(k + 1) * 128],
                                     numer[:, jb * 128:(jb + 1) * 128], idb[:])
                    trs = trs_p.tile([128, JCW], bf, tag="trs")
                    vec.tensor_copy(trs[:], pstr[:])
                    for k in range(4):
                        jb = jc * 4 + k
                        qi = jb % TPQ
                        pe.matmul(psU[:], trs[:, k * 128:(k + 1) * 128],
                                  Vq[q][:, qi * O:(qi + 1) * O],
                                  start=(jb == 0), stop=(jb == NT - 1))
                utile = upd_p.tile([128, O], f32, tag="utile")
                vec.tensor_scalar_mul(utile[:], psU[:], recip[:])
                sync.dma_start(upd_out[osl, :], utile[:])

        # release single-tile pools (LIFO) before the context schedules;
        # a GC-time release would inject boundary insts into the final BIR
        for free in reversed(_keep):
            free()

    nc.compile()
    return nc


_NC_CACHE = {}


def _get_nc():
    if "nc" not in _NC_CACHE:
        _NC_CACHE["nc"] = _build_nc()
    return _NC_CACHE["nc"]


def kernel(**inputs):
    from concourse.bass_utils import run_bass_kernel_spmd

    inp = {k: np.asarray(v, dtype=np.float32) for k, v in inputs.items()}
    nc = _get_nc()

    shared = {k: np.ascontiguousarray(inp[k]) for k in
              ("gene_expr", "spatial_pos", "Wq", "bq", "Wk", "bk", "Wv", "bv",
               "Ws", "bs", "W1", "b1", "W2", "b2", "Wf", "bf")}
    shared["ident"] = np.eye(128, dtype=ml_dtypes.bfloat16)

    in_maps = []
    for c in range(NCORES):
        m = dict(shared)
        m["gene_own"] = np.ascontiguousarray(inp["gene_expr"][c * R:(c + 1) * R])
        m["pos_own"] = np.ascontiguousarray(inp["spatial_pos"][c * R:(c + 1) * R])
        in_maps.append(m)

    res = run_bass_kernel_spmd(nc, in_maps, core_ids=list(range(NCORES)),
                               trace=bool(int(os.environ.get("KERNEL_TRACE", "0"))))
    _NC_CACHE["last_result"] = res
    attn = np.concatenate([r["attn_own"] for r in res.results], axis=0)
    upd = np.concatenate([r["upd_own"] for r in res.results], axis=0)
    return attn, upd
